# revision 48
# baseline (speedup 1.0000x reference)
"""Causal self-attention (GQA, RoPE) on 8 Trainium2 NeuronCores — v9 (bf16).

Sharding: tensor-parallel by KV-head group. Core c owns kv-head c and its 4
query heads, for both batch elements. The host sums the 8 partial
out-projection results (Wout row-sharded), replacing the all-reduce.

Pipeline design (PE ~92% busy; everything else hides under it):
- Phase 1 per 512-token block: qkv^T matmuls (2-bank PSUM ping-pong), then
  K-rope, V-transpose AND Q-rope on the DVE in the same window (DVE is idle
  there); roped Q stays SBUF-resident (no DRAM round-trip).
- Attention per q-block: score matmuls for chunk PAIRS into [128,2,512]
  2-bank PSUM tiles -> ONE exp per pair (halves ACT instruction count) ->
  triangle mask on diag chunks -> denominator (ones-matmul) + AV, software-
  pipelined one pair behind the scores so the PE never waits on the exp.
- Out-projection of block qb-1 is emitted interleaved into attention(qb),
  one quarter per head, so the ACT exp stream never rate-limits the PE.
- Per-(batch, block) K/V tiles for fine-grained cross-stage dependencies.
- w3 loaded as 4 chunk-tiles so first matmuls start early; wout loaded last.

Layouts (per core, s = b*S + pos, SQ = B*S):
  xT    [H, SQ]  bf16   x transposed
  w3    [H, 768] bf16   [Wq(4 heads, pre-scaled by 1/sqrt(hd)) | Wk | Wv]
  wout  [512, H] bf16   Wout rows for this core's 4 q heads
  cosT  [128, S] bf16   cos table transposed
  sinS  [128, S] bf16   sin table, rows 0:64 negated (rotate_half baked in)
  maskT [128, 128] bf16 causal triangle 0/1 mask (q' >= p)
Output: outT [H, SQ] fp16 (partial out-projection, transposed; host sums).

Rejected experiments (measured): fp8 DoubleRow for AV+denominator (fp8 noise
on the softmax numerator is ~3% rel err — doesn't average out because the
output is a weighted mean; and the DoubleRow rhs pairing semantics differ
from the [Ki,Ko,N] AP layout, corrupting sums); fp8 denominator-only (DVE
cast latency on the lp critical path cost more than the PE it saved).
"""
import numpy as np

import concourse.bass as bass
import concourse.mybir as mybir
import concourse.tile as tile
from concourse import bacc
from concourse.masks import make_identity

F32 = mybir.dt.float32
BF16 = mybir.dt.bfloat16
FP16 = mybir.dt.float16
P = 128

N_CORES = 8
CFG = dict(B=2, S=2048, H=4096, HD=128, NQ=4)  # NQ = q heads per core


def build(cfg=CFG, reps=1):
    B, S, H, HD, NQ = cfg["B"], cfg["S"], cfg["H"], cfg["HD"], cfg["NQ"]
    SQ = B * S
    HCH = H // P          # 32 h chunks
    C6 = NQ + 2           # c-tiles: 4 q heads, 1 k, 1 v
    CW = C6 * P           # 768
    NSB = SQ // 512       # 8
    QB = S // 512         # 4 q blocks per batch
    SCH = S // P          # 16 k chunks per batch

    nc = bacc.Bacc("TRN2", target_bir_lowering=False, debug=False,
                   num_devices=N_CORES)
    xT = nc.dram_tensor("xT", [H, SQ], BF16, kind="ExternalInput").ap()
    w3 = nc.dram_tensor("w3", [H, CW], BF16, kind="ExternalInput").ap()
    wout = nc.dram_tensor("wout", [NQ * P, H], BF16, kind="ExternalInput").ap()
    cosT = nc.dram_tensor("cosT", [P, S], BF16, kind="ExternalInput").ap()
    sinS = nc.dram_tensor("sinS", [P, S], BF16, kind="ExternalInput").ap()
    maskT = nc.dram_tensor("maskT", [P, P], BF16, kind="ExternalInput").ap()
    outT = nc.dram_tensor("outT", [H, SQ], FP16, kind="ExternalOutput").ap()

    xT_v = xT.rearrange("(ho p) s -> p ho s", p=P)      # [128, 32, SQ]
    w3_v = w3.rearrange("(ho p) c -> p ho c", p=P)      # [128, 32, 768]
    wout_v = wout.rearrange("(co p) n -> p co n", p=P)  # [128, 4, H]
    outT_v = outT.rearrange("(ho p) s -> p ho s", p=P)  # [128, 32, SQ]

    with tile.TileContext(nc, pool_alloc_mode="queue") as tc:
        with tc.tile_pool(name="dram", bufs=1, space="DRAM") as dram, \
             tc.tile_pool(name="const", bufs=1) as const, \
             tc.tile_pool(name="work", bufs=1) as work, \
             tc.tile_pool(name="psA", bufs=1, space="PSUM") as psA, \
             tc.tile_pool(name="psB", bufs=1, space="PSUM") as psB:

            # constants loaded once per launch (outside the reps loop).
            # w3 as 4 separate tiles (per-chunk dependency!) so the first
            # phase-1 matmuls start as soon as chunk 0 lands; wout last
            # (first needed ~190us in).
            w3_rs = []
            for wi in range(4):
                w3c = const.tile([P, HCH // 4, CW], BF16, name=f"w3r{wi}",
                                 tag=f"w3r{wi}")
                nc.sync.dma_start(w3c[:], w3_v[:, wi * 8:(wi + 1) * 8, :])
                w3_rs.append(w3c)
            mask_t = const.tile([P, P], BF16, name="maskt", tag="maskt")
            nc.sync.dma_start(mask_t[:], maskT[:])
            ones_b = const.tile([P, P], BF16, name="onesb", tag="onesb")
            nc.vector.memset(ones_b[:], 1.0)
            ident_b = const.tile([P, P], BF16, name="identb", tag="identb")
            make_identity(nc, ident_b[:])
            cosT_r = const.tile([P, S], BF16, name="cosr", tag="cosr")
            nc.sync.dma_start(cosT_r[:], cosT[:])
            sinS_r = const.tile([P, S], BF16, name="sinr", tag="sinr")
            nc.sync.dma_start(sinS_r[:], sinS[:])
            wout_r = const.tile([P, NQ, H], BF16, name="woutr", tag="woutr")
            nc.sync.dma_start(wout_r[:], wout_v[:])

            def body(iv=None):
                # per-(batch, 512-block) K/V tiles: fine-grained deps so the
                # first scores of a block don't wait on the whole batch's rope
                kT_r = {}
                v_r = {}
                for b in range(B):
                    for j in range(QB):
                        kT_r[b, j] = work.tile([P, 512], BF16, name=f"kT{b}_{j}",
                                               tag=f"kT{b}_{j}")
                        v_r[b, j] = work.tile([P, 4, HD], BF16, name=f"v{b}_{j}",
                                              tag=f"v{b}_{j}")

                def phase1_block(sb):
                    """qkv^T for s-block sb -> stage tile (returned)."""
                    stage = work.tile([P, C6, 512], BF16, name="stage", tag="stage",
                                    bufs=2)
                    xps = []
                    for xi in range(4):
                        xh = work.tile([P, HCH // 4, 512], BF16, name="xh",
                                     tag="xh", bufs=5)
                        nc.sync.dma_start(
                            xh[:], xT_v[:, xi * 8:(xi + 1) * 8,
                                        sb * 512:(sb + 1) * 512])
                        xps.append(xh)
                    for ci in range(C6):
                        ps = psB.tile([P, 512], F32, name="p1p", tag="pp",
                                      bufs=2)
                        for hc in range(HCH):
                            nc.tensor.matmul(
                                ps[:], w3_rs[hc // 8][:, hc % 8,
                                              ci * P:(ci + 1) * P],
                                xps[hc // 8][:, hc % 8, :],
                                start=(hc == 0), stop=(hc == HCH - 1))
                        if ci % 2 == 0:
                            nc.vector.tensor_copy(stage[:, ci, :], ps[:])
                        else:
                            nc.scalar.copy(stage[:, ci, :], ps[:])
                    return stage

                def q_rope_block(j, stage):
                    """rope the 4 q heads of s-block j (in-batch) from the
                    phase-1 stage tile -> SBUF-resident qro tile. Runs in the
                    phase-1 window where the DVE is otherwise idle."""
                    cs = cosT_r[:, j * 512:j * 512 + 512]
                    sn = sinS_r[:, j * 512:j * 512 + 512]
                    h2 = HD // 2
                    qro = work.tile([P, NQ, 512], BF16, name="qro", tag="qro",
                                  bufs=4)
                    for hh in range(NQ):
                        qrt = work.tile([P, 512], BF16, name="qrt", tag="qrt",
                                      bufs=2)
                        nc.vector.tensor_copy(qrt[:h2, :], stage[h2:, hh, :])
                        nc.vector.tensor_copy(qrt[h2:, :], stage[:h2, hh, :])
                        t1 = work.tile([P, 512], BF16, name="qt1", tag="qt1",
                                     bufs=1)
                        t2 = work.tile([P, 512], BF16, name="qt2", tag="qt2",
                                     bufs=1)
                        nc.vector.tensor_mul(t1[:], stage[:, hh, :], cs)
                        nc.vector.tensor_mul(t2[:], qrt[:], sn)
                        nc.vector.tensor_add(qro[:, hh, :], t1[:], t2[:])
                    return qro

                def kv_rope(b, j, stage):
                    """rope K + transpose V for 512-block j of batch b,
                    reading the phase-1 stage tile directly."""
                    off = j * 512
                    cs = cosT_r[:, off:off + 512]
                    sn = sinS_r[:, off:off + 512]
                    h2 = HD // 2
                    krt = work.tile([P, 512], BF16, name="krt", tag="krt", bufs=2)
                    nc.vector.tensor_copy(krt[:h2, :], stage[h2:, NQ, :])
                    nc.vector.tensor_copy(krt[h2:, :], stage[:h2, NQ, :])
                    kt1 = work.tile([P, 512], BF16, name="kt1", tag="kt1", bufs=1)
                    nc.vector.tensor_mul(kt1[:], stage[:, NQ, :], cs)
                    kt2 = work.tile([P, 512], BF16, name="kt2", tag="kt2", bufs=1)
                    nc.vector.tensor_mul(kt2[:], krt[:], sn)
                    nc.vector.tensor_add(kT_r[b, j][:], kt1[:], kt2[:])
                    for jj in range(4):
                        tps = psB.tile([P, P], BF16, name="vt", tag="pp",
                                       bufs=2)
                        nc.tensor.transpose(
                            tps[:], stage[:, NQ + 1, jj * P:(jj + 1) * P],
                            ident_b[:])
                        nc.vector.tensor_copy(v_r[b, j][:, jj, :], tps[:])

                def attention_head(b, qb, h, qr, att):
                    """scores -> exp (one per chunk PAIR) -> mask ->
                    denom+AV -> att[:, h, :]."""
                    nch = (qb + 1) * 4
                    lp = psA.tile([P, 512], F32, name="lp", tag="lp", bufs=1)
                    av = psA.tile([P, 512], F32, name="av", tag="av", bufs=1)

                    def emit_lp_av(kp):
                        ka, kb = 2 * kp, 2 * kp + 1
                        va, vb = ka - (nch - 4), kb - (nch - 4)
                        c0a = va * P if va > 0 else 0
                        c0b = vb * P if vb > 0 else 0
                        pt = pts[kp]
                        nc.tensor.matmul(
                            lp[:, c0a:], ones_b[:], pt[:, 0, c0a:],
                            start=(ka == 0), stop=False)
                        nc.tensor.matmul(
                            lp[:, c0b:], ones_b[:], pt[:, 1, c0b:],
                            start=False, stop=(kb == nch - 1))
                        nc.tensor.matmul(
                            av[:, c0a:], v_r[b, ka // 4][:, ka % 4, :],
                            pt[:, 0, c0a:], start=(ka == 0), stop=False)
                        nc.tensor.matmul(
                            av[:, c0b:], v_r[b, kb // 4][:, kb % 4, :],
                            pt[:, 1, c0b:], start=False, stop=(kb == nch - 1))

                    pts = {}
                    for kp in range(nch // 2):
                        ka, kb = 2 * kp, 2 * kp + 1
                        va, vb = ka - (nch - 4), kb - (nch - 4)
                        # true valid col starts; scores for the pair are both
                        # computed from the pair's min col0 so one exp call
                        # covers both banks with real (finite) scores.
                        c0a = va * P if va > 0 else 0
                        c0b = vb * P if vb > 0 else 0
                        sc = psA.tile([P, 2, 512], F32, name="sc", tag="sc",
                                      bufs=2)
                        nc.tensor.matmul(
                            sc[:, 0, c0a:], kT_r[b, ka // 4][:, (ka % 4) * P:
                                                             (ka % 4 + 1) * P],
                            qr[:, h, c0a:], start=True, stop=True)
                        nc.tensor.matmul(
                            sc[:, 1, c0a:], kT_r[b, kb // 4][:, (kb % 4) * P:
                                                             (kb % 4 + 1) * P],
                            qr[:, h, c0a:], start=True, stop=True)
                        pt = work.tile([P, 2, 512], BF16, name="pt", tag="pt",
                                     bufs=3)
                        nc.scalar.activation(
                            pt[:, :, c0a:], sc[:, :, c0a:],
                            mybir.ActivationFunctionType.Exp)
                        if va >= 0:
                            nc.vector.tensor_mul(
                                pt[:, 0, c0a:c0a + P],
                                pt[:, 0, c0a:c0a + P], mask_t[:])
                        if vb >= 0:
                            nc.vector.tensor_mul(
                                pt[:, 1, c0b:c0b + P],
                                pt[:, 1, c0b:c0b + P], mask_t[:])
                        pts[kp] = pt
                        # software pipeline: lp/av trail the scores by one
                        # pair so the PE never sits right behind the exp.
                        if kp > 0:
                            emit_lp_av(kp - 1)
                    emit_lp_av(nch // 2 - 1)
                    rec = work.tile([P, 512], F32, name="rec", tag="rec", bufs=1)
                    nc.vector.reciprocal_approx_fast(rec[:], lp[:])
                    nc.vector.tensor_mul(att[:, h, :], av[:], rec[:])

                def out_proj_part(b, qb, att, part):
                    """one quarter (8 ht) of the out-projection for block qb."""
                    sb = b * QB + qb
                    NHP = H // P // 2
                    for hp in range(part * NHP // 4, (part + 1) * NHP // 4):
                        ost = work.tile([P, 2, 512], FP16, name="ost", tag="ost",
                                      bufs=3)
                        for hi in range(2):
                            ht = 2 * hp + hi
                            o3 = psB.tile([P, 512], F32, name="o3", tag="pp",
                                          bufs=2)
                            for ci in range(NQ):
                                nc.tensor.matmul(
                                    o3[:], wout_r[:, ci, ht * P:(ht + 1) * P],
                                    att[:, ci, :],
                                    start=(ci == 0), stop=(ci == NQ - 1))
                            if ht % 2 == 0:
                                nc.scalar.copy(ost[:, hi, :], o3[:])
                            else:
                                nc.vector.tensor_copy(ost[:, hi, :], o3[:])
                        nc.sync.dma_start(
                            outT_v[:, 2 * hp:2 * hp + 2,
                                   sb * 512:(sb + 1) * 512],
                            ost[:])

                for b in range(B):
                    qros = []
                    for j in range(QB):
                        stage = phase1_block(b * QB + j)
                        kv_rope(b, j, stage)
                        qros.append(q_rope_block(j, stage))
                    # out_proj(qb-1) is emitted interleaved INTO attention(qb),
                    # one quarter after each head: the PE chews on out-proj
                    # matmuls while attention's exp/mask pipeline fills, and
                    # the ACT exp stream never rate-limits the PE.
                    att_prev = None
                    for qb in range(QB):
                        att = work.tile([P, NQ, 512], BF16, name="att", tag="att",
                                      bufs=2)
                        for h in range(NQ):
                            attention_head(b, qb, h, qros[qb], att)
                            if att_prev is not None:
                                out_proj_part(b, qb - 1, att_prev, h)
                        att_prev = att
                    for part in range(4):
                        out_proj_part(b, QB - 1, att_prev, part)

            if reps == 1:
                body()
            else:
                with tc.For_i(0, reps, 1) as iv:
                    body(iv)
    return nc


def host_inputs(x, cos, sin, Wqkv, Wout, cfg=CFG):
    """Build the 8 per-core input maps from the full-problem inputs."""
    import ml_dtypes
    bf16 = ml_dtypes.bfloat16
    B, S, H, HD, NQ = cfg["B"], cfg["S"], cfg["H"], cfg["HD"], cfg["NQ"]
    SQ = B * S
    NH = NQ * N_CORES
    scale = 1.0 / np.sqrt(HD)

    x = np.asarray(x, dtype=np.float32)
    cos = np.asarray(cos, dtype=np.float32)
    sin = np.asarray(sin, dtype=np.float32)
    Wqkv = np.asarray(Wqkv, dtype=np.float32)
    Wout = np.asarray(Wout, dtype=np.float32)

    xT_b = np.ascontiguousarray(x.reshape(SQ, H).T).astype(bf16)
    cosT2 = np.ascontiguousarray(cos.T).astype(bf16)
    sinT = sin.T
    sinS2 = np.concatenate([-sinT[:HD // 2], sinT[HD // 2:]], axis=0)
    sinS2 = np.ascontiguousarray(sinS2).astype(bf16)
    qv = np.arange(P)
    pv = np.arange(P)
    mask = (qv[None, :] >= pv[:, None]).astype(bf16)

    in_maps = []
    for c in range(N_CORES):
        wq = Wqkv[:, c * NQ * HD:(c + 1) * NQ * HD] * scale
        wk = Wqkv[:, NH * HD + c * HD: NH * HD + (c + 1) * HD]
        wv = Wqkv[:, NH * HD + N_CORES * HD + c * HD:
                  NH * HD + N_CORES * HD + (c + 1) * HD]
        w3c = np.concatenate([wq, wk, wv], axis=1).astype(bf16)
        woutc = Wout[c * NQ * HD:(c + 1) * NQ * HD, :].astype(bf16)
        in_maps.append({
            "xT": xT_b, "w3": w3c, "wout": woutc,
            "cosT": cosT2, "sinS": sinS2, "maskT": mask,
        })
    return in_maps


class _Runner:
    """Compiled-kernel runner over the axon PJRT path (kept for re-invocation)."""

    def __init__(self, nc, n_cores):
        import jax
        from jax.sharding import Mesh, PartitionSpec
        from jax.experimental.shard_map import shard_map
        from concourse.bass2jax import (
            _bass_exec_p, partition_id_tensor, install_neuronx_cc_hook)
        install_neuronx_cc_hook()
        self.nc = nc
        self.jax = jax
        self.n_cores = n_cores
        partition_name = nc.partition_id_tensor.name if nc.partition_id_tensor else None
        in_names, out_names, out_avals, zero_outs = [], [], [], []
        for alloc in nc.m.functions[0].allocations:
            if not isinstance(alloc, mybir.MemoryLocationSet):
                continue
            name = alloc.memorylocations[0].name
            if alloc.kind == "ExternalInput":
                if name != partition_name:
                    in_names.append(name)
            elif alloc.kind == "ExternalOutput":
                shape = tuple(alloc.tensor_shape)
                dtype = mybir.dt.np(alloc.dtype)
                out_avals.append(jax.core.ShapedArray(shape, dtype))
                out_names.append(name)
                zero_outs.append(np.zeros(shape, dtype))
        self.in_names = in_names[:]
        self.out_names, self.out_avals, self.zero_outs = out_names, out_avals, zero_outs
        self.n_params = len(in_names)
        all_names = in_names + out_names
        if partition_name is not None:
            all_names.append(partition_name)

        def _body(*args):
            operands = list(args)
            if partition_name is not None:
                operands.append(partition_id_tensor())
            outs = _bass_exec_p.bind(
                *operands, out_avals=tuple(out_avals), in_names=tuple(all_names),
                out_names=tuple(out_names), lowering_input_output_aliases=(),
                sim_require_finite=True, sim_require_nnan=True, nc=nc)
            return tuple(outs)

        devices = jax.devices()[:n_cores]
        self.mesh = Mesh(np.asarray(devices), ("core",))
        specs_in = (PartitionSpec("core"),) * (self.n_params + len(out_names))
        specs_out = (PartitionSpec("core"),) * len(out_names)
        self.sharded = jax.jit(
            shard_map(_body, mesh=self.mesh, in_specs=specs_in,
                      out_specs=specs_out, check_rep=False),
            keep_unused=True)
        self._dev_args = None

    def stage(self, in_maps):
        import jax
        from jax.sharding import PartitionSpec
        per_core = [[np.asarray(m[n]) for n in self.in_names] for m in in_maps]
        concat = [np.concatenate([per_core[c][i] for c in range(self.n_cores)], axis=0)
                  for i in range(self.n_params)]
        concat += [np.zeros((self.n_cores * z.shape[0], *z.shape[1:]), z.dtype)
                   for z in self.zero_outs]
        sh = jax.sharding.NamedSharding(self.mesh, PartitionSpec("core"))
        self._dev_args = [jax.device_put(a, sh) for a in concat]
        jax.block_until_ready(self._dev_args)

    def execute(self):
        out = self.sharded(*self._dev_args)
        self.jax.block_until_ready(out)
        return out

    def results(self, out):
        return [
            {n: np.asarray(out[i]).reshape(self.n_cores, *self.out_avals[i].shape)[c]
             for i, n in enumerate(self.out_names)}
            for c in range(self.n_cores)
        ]


_cached = {}


def _get_runner(reps=1):
    key = reps
    if key not in _cached:
        nc = build(CFG, reps=reps)
        nc.compile()
        _cached[key] = _Runner(nc, N_CORES)
    return _cached[key]


def kernel(x, cos, sin, Wqkv, Wout):
    cfg = CFG
    B, S, H = cfg["B"], cfg["S"], cfg["H"]
    runner = _get_runner(reps=1)
    in_maps = host_inputs(x, cos, sin, Wqkv, Wout, cfg)
    runner.stage(in_maps)
    out = runner.execute()
    results = runner.results(out)
    acc = np.zeros((B * S, H), np.float32)
    for c in range(N_CORES):
        acc += results[c]["outT"].T.astype(np.float32)
    return acc.reshape(B, S, H).astype(np.float32)

